# revision 1
# baseline (speedup 1.0000x reference)
"""Multi-head attention layer (N=4, L=S=2048, D=1024, H=16) on 8 TRN2 NeuronCores.

Sharding: 8 cores = 4 batches x 2 query-halves (head-dim kept local, so no
collectives are needed: each core computes Q projection for its 1024 query
rows, K/V projections for the full 2048 keys of its batch, all 16 heads of
attention, and the output projection for its rows). Host shards/gathers.

Per-core data layout (host-prepared, bf16):
  xq [128, 8, 1024]  xq[p,t,l] = queries[n, l0+l, t*128+p]   (transposed)
  xk [128, 8, 2048]  keys[n].T, same packing
  xv [128, 8, 2048]  values[n].T
  wq/wk/wv/wo [128, 8, 1024]  w[p,t,d] = W[t*128+p, d]
  bq/bk [128, 8] f32; bv [64, 16] f32; bo [128, 1024] f32 (pre-broadcast)
  out [1024, 1024] f32 (natural layout)

Compute (per core):
  Q^T = Wq^T @ queries^T (+bq)  -> [128, 8, 1024] bf16 (2 heads per d-tile)
  K^T similarly [128, 8, 2048]; V natural [s, d] scattered into V_aug
  [128 (s-tile), 16 (s-tiles), 16*65] with a ones column per head.
  Per (head, l-block of 512): scores^T[s,l] accumulated per s-tile of 128,
  exp(0.125*scores) on ScalarE (PSUM->bf16), PV matmul with the 65-wide
  V_aug (row 64 of PSUM = softmax denominator), normalize via reciprocal +
  partition_broadcast, add bv (attn rows sum to 1 so P@(V+bv) = P@V + bv).
  Output projection back to natural [l, d] + bo, DMA out.
"""

import numpy as np
import ml_dtypes

import concourse.bass as bass
import concourse.mybir as mybir
import concourse.tile as tile
from concourse import bacc
from concourse.bass_utils import run_bass_kernel_spmd

BF16 = mybir.dt.bfloat16
F32 = mybir.dt.float32
ALU = mybir.AluOpType
ACTF = mybir.ActivationFunctionType

N, L, S, D, H, E = 4, 2048, 2048, 1024, 16, 64
LQ = 1024           # query rows per core
N_CORES = 8

_nc_cache = None
last_results = None  # BassKernelResults of the most recent run (for test harness)


def _build():
    nc = bacc.Bacc(None, target_bir_lowering=False)

    xq = nc.declare_dram_parameter("xq", [128, 8, LQ], BF16, isOutput=False)
    xk = nc.declare_dram_parameter("xk", [128, 8, S], BF16, isOutput=False)
    xv = nc.declare_dram_parameter("xv", [128, 8, S], BF16, isOutput=False)
    wq = nc.declare_dram_parameter("wq", [128, 8, D], BF16, isOutput=False)
    wk = nc.declare_dram_parameter("wk", [128, 8, D], BF16, isOutput=False)
    wv = nc.declare_dram_parameter("wv", [128, 8, D], BF16, isOutput=False)
    wo = nc.declare_dram_parameter("wo", [128, 8, D], BF16, isOutput=False)
    bq = nc.declare_dram_parameter("bq", [128, 8], F32, isOutput=False)
    bk = nc.declare_dram_parameter("bk", [128, 8], F32, isOutput=False)
    bv = nc.declare_dram_parameter("bv", [64, 16], F32, isOutput=False)
    bo = nc.declare_dram_parameter("bo", [128, D], F32, isOutput=False)
    out = nc.declare_dram_parameter("out", [LQ, D], F32, isOutput=True)

    with tile.TileContext(nc) as tc:
        with tc.tile_pool(name="const", bufs=1) as cpool, \
             tc.tile_pool(name="pers", bufs=1) as ppool, \
             tc.tile_pool(name="stage", bufs=2) as spool, \
             tc.tile_pool(name="work", bufs=2) as wpool, \
             tc.tile_pool(name="expp", bufs=4) as epool, \
             tc.tile_pool(name="psum", bufs=2, space="PSUM") as psum:

            # --- constants.  wq/wv share a slot (wv loads after Q proj
            # finishes), wk/wo share likewise: halves weight SBUF footprint.
            wq_t = cpool.tile([128, 8, D], BF16, tag="w_a")
            wk_t = cpool.tile([128, 8, D], BF16, tag="w_b")
            nc.sync.dma_start(wq_t[:], wq[:])
            nc.sync.dma_start(wk_t[:], wk[:])
            bq_t = cpool.tile([128, 8], F32, tag="bq")
            bk_t = cpool.tile([128, 8], F32, tag="bk")
            bv_t = cpool.tile([64, 16], F32, tag="bv")
            bo_t = cpool.tile([128, D], F32, tag="bo")
            nc.sync.dma_start(bq_t[:], bq[:])
            nc.sync.dma_start(bk_t[:], bk[:])
            nc.sync.dma_start(bv_t[:], bv[:])
            nc.sync.dma_start(bo_t[:], bo[:])

            # --- persistent intermediates
            qT = ppool.tile([128, 8, LQ], BF16, tag="qT")    # Q^T, 2 heads/tile
            kT = ppool.tile([128, 8, S], BF16, tag="kT")     # K^T
            vaug = ppool.tile([128, 16, 16 * 65], BF16, tag="vaug")  # V+ones
            oT = ppool.tile([128, 8, LQ], BF16, tag="oT")    # attn out^T

            # ones columns of V_aug (col 64 of each head block)
            for st in range(16):
                v3 = vaug[:, st].rearrange("p (h e) -> p h e", e=65)
                nc.vector.memset(v3[:, :, 64:65], 1.0)

            # ---------------- Q projection ----------------
            for lb in range(2):
                sg = spool.tile([128, 8, 512], BF16, tag="stage")
                nc.sync.dma_start(sg[:], xq[:, :, lb * 512:(lb + 1) * 512])
                for dt in range(8):
                    ps = psum.tile([128, 512], F32, tag="proj")
                    for ct in range(8):
                        nc.tensor.matmul(
                            ps[:], wq_t[:, ct, dt * 128:(dt + 1) * 128],
                            sg[:, ct, :], start=(ct == 0), stop=(ct == 7))
                    nc.vector.tensor_scalar_add(
                        qT[:, dt, lb * 512:(lb + 1) * 512], ps[:],
                        bq_t[:, dt:dt + 1])

            # ---------------- K projection ----------------
            for sb in range(4):
                sg = spool.tile([128, 8, 512], BF16, tag="stage")
                nc.sync.dma_start(sg[:], xk[:, :, sb * 512:(sb + 1) * 512])
                for dt in range(8):
                    ps = psum.tile([128, 512], F32, tag="proj")
                    for ct in range(8):
                        nc.tensor.matmul(
                            ps[:], wk_t[:, ct, dt * 128:(dt + 1) * 128],
                            sg[:, ct, :], start=(ct == 0), stop=(ct == 7))
                    nc.vector.tensor_scalar_add(
                        kT[:, dt, sb * 512:(sb + 1) * 512], ps[:],
                        bk_t[:, dt:dt + 1])

            # ---------------- V projection (natural layout) ----------------
            wv_t = cpool.tile([128, 8, D], BF16, tag="w_a")  # reuses wq slot
            nc.sync.dma_start(wv_t[:], wv[:])
            for sb in range(4):
                sg = spool.tile([128, 8, 512], BF16, tag="stage")
                nc.sync.dma_start(sg[:], xv[:, :, sb * 512:(sb + 1) * 512])
                for stl in range(4):          # s-tiles of 128 within block
                    st = sb * 4 + stl
                    for db in range(2):       # d blocks of 512 = 8 heads
                        ps = psum.tile([128, 512], F32, tag="proj")
                        for ct in range(8):
                            nc.tensor.matmul(
                                ps[:], sg[:, ct, stl * 128:(stl + 1) * 128],
                                wv_t[:, ct, db * 512:(db + 1) * 512],
                                start=(ct == 0), stop=(ct == 7))
                        v3 = vaug[:, st].rearrange("p (h e) -> p h e", e=65)
                        nc.vector.tensor_copy(
                            v3[:, db * 8:(db + 1) * 8, 0:64],
                            ps[:].rearrange("p (h e) -> p h e", e=64))

            # ---------------- attention ----------------
            wo_t = cpool.tile([128, 8, D], BF16, tag="w_b")  # reuses wk slot
            nc.sync.dma_start(wo_t[:], wo[:])
            for h in range(16):
                pb = (h % 2) * 64       # partition base of this head
                dt = h // 2
                for lb in range(2):
                    qh = qT[pb:pb + 64, dt, lb * 512:(lb + 1) * 512]
                    po = psum.tile([128, 512], F32, tag="po")
                    for st in range(16):
                        ss = psum.tile([128, 512], F32, tag="scores")
                        nc.tensor.matmul(
                            ss[:], kT[pb:pb + 64, dt, st * 128:(st + 1) * 128],
                            qh, start=True, stop=True)
                        ep = epool.tile([128, 512], BF16, tag="ep")
                        nc.scalar.activation(ep[:], ss[:], ACTF.Exp, scale=0.125)
                        nc.tensor.matmul(
                            po[0:65, :], vaug[:, st, h * 65:(h + 1) * 65],
                            ep[:], start=(st == 0), stop=(st == 15))
                    # normalize: row 64 = sum of exp
                    rec = wpool.tile([128, 512], F32, tag="rec")
                    nc.vector.reciprocal(rec[64:65, :], po[64:65, :])
                    rec0 = wpool.tile([1, 512], F32, tag="rec0")
                    nc.sync.dma_start(rec0[0:1, :], rec[64:65, :])
                    recb = wpool.tile([64, 512], F32, tag="recb")
                    nc.gpsimd.partition_broadcast(recb[:], rec0[0:1, :])
                    if pb == 0:
                        dst = oT[0:64, dt, lb * 512:(lb + 1) * 512]
                        nc.vector.tensor_tensor(dst, po[0:64, :], recb[:],
                                                ALU.mult)
                        nc.vector.tensor_scalar_add(dst, dst, bv_t[:, h:h + 1])
                    else:
                        tmp = wpool.tile([64, 512], BF16, tag="otmp")
                        nc.vector.tensor_tensor(tmp[:], po[0:64, :], recb[:],
                                                ALU.mult)
                        nc.vector.tensor_scalar_add(tmp[:], tmp[:],
                                                    bv_t[:, h:h + 1])
                        nc.sync.dma_start(
                            oT[64:128, dt, lb * 512:(lb + 1) * 512], tmp[:])

            # ---------------- output projection ----------------
            for lt in range(8):
                ob = wpool.tile([128, D], F32, tag="outsb")
                for db in range(2):
                    ps = psum.tile([128, 512], F32, tag="proj")
                    for ct in range(8):
                        nc.tensor.matmul(
                            ps[:], oT[:, ct, lt * 128:(lt + 1) * 128],
                            wo_t[:, ct, db * 512:(db + 1) * 512],
                            start=(ct == 0), stop=(ct == 7))
                    nc.vector.tensor_add(ob[:, db * 512:(db + 1) * 512],
                                         ps[:], bo_t[:, db * 512:(db + 1) * 512])
                nc.sync.dma_start(out[lt * 128:(lt + 1) * 128, :], ob[:])

    nc.compile()
    return nc


def _pack_kxm(w):
    # [K, M] f32 -> [128, K//128, M] bf16
    k, m = w.shape
    return np.ascontiguousarray(
        w.reshape(k // 128, 128, m).transpose(1, 0, 2)).astype(ml_dtypes.bfloat16)


def kernel(queries, keys, values, Wq, bq, Wk, bk, Wv, bv, Wo, bo):
    global _nc_cache, last_results
    queries = np.asarray(queries, dtype=np.float32)
    keys = np.asarray(keys, dtype=np.float32)
    values = np.asarray(values, dtype=np.float32)

    if _nc_cache is None:
        _nc_cache = _build()
    nc = _nc_cache

    w_packed = {
        "wq": _pack_kxm(np.asarray(Wq, np.float32)),
        "wk": _pack_kxm(np.asarray(Wk, np.float32)),
        "wv": _pack_kxm(np.asarray(Wv, np.float32)),
        "wo": _pack_kxm(np.asarray(Wo, np.float32)),
        "bq": np.ascontiguousarray(
            np.asarray(bq, np.float32).reshape(8, 128).T),
        "bk": np.ascontiguousarray(
            np.asarray(bk, np.float32).reshape(8, 128).T),
        "bv": np.ascontiguousarray(
            np.asarray(bv, np.float32).reshape(16, 64).T),
        "bo": np.ascontiguousarray(
            np.broadcast_to(np.asarray(bo, np.float32), (128, D))),
    }

    in_maps = []
    for c in range(N_CORES):
        n, half = c // 2, c % 2
        m = dict(w_packed)
        m["xq"] = _pack_kxm(
            np.ascontiguousarray(queries[n, half * LQ:(half + 1) * LQ, :].T))
        m["xk"] = _pack_kxm(np.ascontiguousarray(keys[n].T))
        m["xv"] = _pack_kxm(np.ascontiguousarray(values[n].T))
        in_maps.append(m)

    last_results = run_bass_kernel_spmd(nc, in_maps, list(range(N_CORES)))

    full = np.empty((N, L, D), np.float32)
    for c in range(N_CORES):
        n, half = c // 2, c % 2
        full[n, half * LQ:(half + 1) * LQ, :] = last_results.results[c]["out"]
    return full


# revision 3
# speedup vs baseline: 1.4391x; 1.4391x over previous
"""Multi-head attention layer (N=4, L=S=2048, D=1024, H=16) on 8 TRN2 NeuronCores.

Sharding: 8 cores = 4 batches x 2 query-halves (heads kept local, so no
collectives: each core computes Q projection for its 1024 query rows, K/V
projections for the full 2048 keys of its batch, all 16 heads of attention,
and the output projection for its rows). Host shards/gathers.

Per-core data layout (host-prepared, bf16):
  xq [128, 8, 1024]  xq[p,t,l] = queries[n, l0+l, t*128+p]   (transposed)
  xk/xv [128, 8, 2048]  keys[n].T / values[n].T, same packing
  wq/wk/wv/wo [128, 8, 1024]  w[p,t,d] = W[t*128+p, d]
  bq/bk [128, 8] f32; bv [64, 16] f32; bo [128, 1024] f32 (pre-broadcast)
  out [1024, 1024] f32 (natural layout)

Pipeline notes: attention is ScalarE(exp)-paced — exp runs 1 elem/lane/cycle
at 1.2 GHz vs warm PE at 2.4 GHz, so exp is batched over [128,1024] PSUM
groups (amortizing the 352-cycle ACT overhead) and independent projection
matmul groups are interleaved into the attention emission order as PE
filler, keeping the PE dense so HAM doesn't re-throttle it to 1.2 GHz.
The softmax denominator comes free from a ones-column appended to V (PV
matmul has 65 output rows; row 64 = sum of exp). attn rows sum to 1, so
V's bias is added after normalization (P@(V+bv) = P@V + bv).
"""

import numpy as np
import ml_dtypes

import concourse.bass as bass
import concourse.mybir as mybir
import concourse.tile as tile
from concourse import bacc
from concourse.bass_utils import run_bass_kernel_spmd

BF16 = mybir.dt.bfloat16
F32 = mybir.dt.float32
ALU = mybir.AluOpType
ACTF = mybir.ActivationFunctionType

N, L, S, D, H, E = 4, 2048, 2048, 1024, 16, 64
LQ = 1024
N_CORES = 8

_nc_cache = None
last_results = None


def _build():
    nc = bacc.Bacc(None, target_bir_lowering=False)

    xq = nc.declare_dram_parameter("xq", [128, 8, LQ], BF16, isOutput=False)
    xk = nc.declare_dram_parameter("xk", [128, 8, S], BF16, isOutput=False)
    xv = nc.declare_dram_parameter("xv", [128, 8, S], BF16, isOutput=False)
    wq = nc.declare_dram_parameter("wq", [128, 8, D], BF16, isOutput=False)
    wk = nc.declare_dram_parameter("wk", [128, 8, D], BF16, isOutput=False)
    wv = nc.declare_dram_parameter("wv", [128, 8, D], BF16, isOutput=False)
    wo = nc.declare_dram_parameter("wo", [128, 8, D], BF16, isOutput=False)
    bq = nc.declare_dram_parameter("bq", [128, 8], F32, isOutput=False)
    bk = nc.declare_dram_parameter("bk", [128, 8], F32, isOutput=False)
    bv = nc.declare_dram_parameter("bv", [64, 16], F32, isOutput=False)
    bo = nc.declare_dram_parameter("bo", [128, D], F32, isOutput=False)
    out = nc.declare_dram_parameter("out", [LQ, D], F32, isOutput=True)

    with tile.TileContext(nc) as tc:
        with tc.tile_pool(name="const", bufs=1) as cpool, \
             tc.tile_pool(name="pers", bufs=1) as ppool, \
             tc.tile_pool(name="stage", bufs=2) as spool, \
             tc.tile_pool(name="work", bufs=2) as wpool, \
             tc.tile_pool(name="expp", bufs=3) as epool, \
             tc.tile_pool(name="psum", bufs=2, space="PSUM") as psum:

            # weights: wq's slot is reused by wo after the Q projection.
            wq_t = cpool.tile([128, 8, D], BF16, tag="w_a")
            wk_t = cpool.tile([128, 8, D], BF16, tag="w_b")
            wv_t = cpool.tile([128, 8, D], BF16, tag="w_c")
            nc.sync.dma_start(wq_t[:], wq[:])
            nc.sync.dma_start(wk_t[:], wk[:])
            nc.sync.dma_start(wv_t[:], wv[:])
            bq_t = cpool.tile([128, 8], F32, tag="bq")
            bk_t = cpool.tile([128, 8], F32, tag="bk")
            bv_t = cpool.tile([64, 16], F32, tag="bv")
            bo_t = cpool.tile([128, D], F32, tag="bo")
            nc.sync.dma_start(bq_t[:], bq[:])
            nc.sync.dma_start(bk_t[:], bk[:])
            nc.sync.dma_start(bv_t[:], bv[:])
            nc.sync.dma_start(bo_t[:], bo[:])

            qT = ppool.tile([128, 8, LQ], BF16, tag="qT")
            kT = ppool.tile([128, 8, S], BF16, tag="kT")
            vaug = ppool.tile([128, 16, 16 * 65], BF16, tag="vaug")
            oT = ppool.tile([128, 8, LQ], BF16, tag="oT")

            for st in range(16):
                v3 = vaug[:, st].rearrange("p (h e) -> p h e", e=65)
                nc.vector.memset(v3[:, :, 64:65], 1.0)

            # one projection PSUM group: 8 accumulating matmuls + epilogue
            def proj_group(w_t, sg_t, dt, dst, bias):
                ps = psum.tile([128, 512], F32, tag="proj")
                for ct in range(8):
                    nc.tensor.matmul(ps[:], w_t[:, ct, dt * 128:(dt + 1) * 128],
                                     sg_t[:, ct, :], start=(ct == 0),
                                     stop=(ct == 7))
                nc.vector.tensor_scalar_add(dst, ps[:], bias)

            def q_proj(lb):
                sg = spool.tile([128, 8, 512], BF16, tag="stage")
                nc.sync.dma_start(sg[:], xq[:, :, lb * 512:(lb + 1) * 512])
                return [
                    (lambda dt=dt, sg=sg, lb=lb: proj_group(
                        wq_t, sg, dt, qT[:, dt, lb * 512:(lb + 1) * 512],
                        bq_t[:, dt:dt + 1]))
                    for dt in range(8)
                ]

            def k_proj(sb):
                sg = spool.tile([128, 8, 512], BF16, tag="stage")
                nc.sync.dma_start(sg[:], xk[:, :, sb * 512:(sb + 1) * 512])
                return [
                    (lambda dt=dt, sg=sg, sb=sb: proj_group(
                        wk_t, sg, dt, kT[:, dt, sb * 512:(sb + 1) * 512],
                        bk_t[:, dt:dt + 1]))
                    for dt in range(8)
                ]

            def v_proj_group(sg_t, stl, st, db):
                ps = psum.tile([128, 512], F32, tag="proj")
                for ct in range(8):
                    nc.tensor.matmul(ps[:], sg_t[:, ct, stl * 128:(stl + 1) * 128],
                                     wv_t[:, ct, db * 512:(db + 1) * 512],
                                     start=(ct == 0), stop=(ct == 7))
                v3 = vaug[:, st].rearrange("p (h e) -> p h e", e=65)
                nc.vector.tensor_copy(
                    v3[:, db * 8:(db + 1) * 8, 0:64],
                    ps[:].rearrange("p (h e) -> p h e", e=64))

            def o_proj_group(lt, db):
                ps = psum.tile([128, 512], F32, tag="proj")
                for ct in range(8):
                    nc.tensor.matmul(ps[:], oT[:, ct, lt * 128:(lt + 1) * 128],
                                     wo_t[:, ct, db * 512:(db + 1) * 512],
                                     start=(ct == 0), stop=(ct == 7))
                ob = wpool.tile([128, 512], F32, tag="outsb")
                nc.vector.tensor_add(ob[:], ps[:],
                                     bo_t[:, db * 512:(db + 1) * 512])
                nc.sync.dma_start(
                    out[lt * 128:(lt + 1) * 128, db * 512:(db + 1) * 512], ob[:])

            # ---- up-front projections: Q(lb0), K(all), V(all) ----
            for g in q_proj(0):
                g()
            for sb in range(4):
                for g in k_proj(sb):
                    g()
            for sb in range(4):
                sg = spool.tile([128, 8, 512], BF16, tag="stage")
                nc.sync.dma_start(sg[:], xv[:, :, sb * 512:(sb + 1) * 512])
                for stl in range(4):
                    for db in range(2):
                        v_proj_group(sg, stl, sb * 4 + stl, db)

            # ---- attention, lb-outer; filler groups interleaved ----
            def attention(h, lb, filler):
                pb = (h % 2) * 64
                dt = h // 2
                qh = qT[pb:pb + 64, dt, lb * 512:(lb + 1) * 512]
                po = psum.tile([128, 512], F32, tag="po")
                for sg_i in range(8):       # s-groups of 2 tiles (256 rows)
                    ps2 = psum.tile([128, 1024], F32, tag="sc2")
                    for k in range(2):
                        st = sg_i * 2 + k
                        nc.tensor.matmul(
                            ps2[:, k * 512:(k + 1) * 512],
                            kT[pb:pb + 64, dt, st * 128:(st + 1) * 128],
                            qh, start=True, stop=True)
                    ep = epool.tile([128, 1024], BF16, tag="ep")
                    nc.scalar.activation(ep[:], ps2[:], ACTF.Exp, scale=0.125)
                    if sg_i == 3 and filler:
                        filler.pop(0)()     # PE filler inside the s-loop
                    for k in range(2):
                        st = sg_i * 2 + k
                        nc.tensor.matmul(
                            po[0:65, :], vaug[:, st, h * 65:(h + 1) * 65],
                            ep[:, k * 512:(k + 1) * 512],
                            start=(st == 0), stop=(st == 15))
                rec = wpool.tile([128, 512], F32, tag="rec")
                nc.vector.reciprocal(rec[64:65, :], po[64:65, :])
                rec0 = wpool.tile([1, 512], F32, tag="rec0")
                nc.sync.dma_start(rec0[0:1, :], rec[64:65, :])
                recb = wpool.tile([64, 512], F32, tag="recb")
                nc.gpsimd.partition_broadcast(recb[:], rec0[0:1, :])
                if pb == 0:
                    dst = oT[0:64, dt, lb * 512:(lb + 1) * 512]
                    nc.vector.tensor_tensor(dst, po[0:64, :], recb[:], ALU.mult)
                    nc.vector.tensor_scalar_add(dst, dst, bv_t[:, h:h + 1])
                else:
                    tmp = wpool.tile([64, 512], BF16, tag="otmp")
                    nc.vector.tensor_tensor(tmp[:], po[0:64, :], recb[:],
                                            ALU.mult)
                    nc.vector.tensor_scalar_add(tmp[:], tmp[:], bv_t[:, h:h + 1])
                    nc.sync.dma_start(
                        oT[64:128, dt, lb * 512:(lb + 1) * 512], tmp[:])

            # lb=0 phase: filler = Q projection for lb=1
            filler = q_proj(1)
            for h in range(16):
                attention(h, 0, filler)
            while filler:
                filler.pop(0)()
            wo_t = cpool.tile([128, 8, D], BF16, tag="w_a")  # reuses wq slot
            nc.sync.dma_start(wo_t[:], wo[:])

            # lb=1 phase: filler = output projection for l-tiles 0..3
            filler = [
                (lambda lt=lt, db=db: o_proj_group(lt, db))
                for lt in range(4) for db in range(2)
            ]
            for h in range(16):
                attention(h, 1, filler)
            while filler:
                filler.pop(0)()

            for lt in range(4, 8):
                for db in range(2):
                    o_proj_group(lt, db)

    nc.compile()
    return nc


def _pack_kxm(w):
    k, m = w.shape
    return np.ascontiguousarray(
        w.reshape(k // 128, 128, m).transpose(1, 0, 2)).astype(ml_dtypes.bfloat16)


def kernel(queries, keys, values, Wq, bq, Wk, bk, Wv, bv, Wo, bo):
    global _nc_cache, last_results
    queries = np.asarray(queries, dtype=np.float32)
    keys = np.asarray(keys, dtype=np.float32)
    values = np.asarray(values, dtype=np.float32)

    if _nc_cache is None:
        _nc_cache = _build()
    nc = _nc_cache

    w_packed = {
        "wq": _pack_kxm(np.asarray(Wq, np.float32)),
        "wk": _pack_kxm(np.asarray(Wk, np.float32)),
        "wv": _pack_kxm(np.asarray(Wv, np.float32)),
        "wo": _pack_kxm(np.asarray(Wo, np.float32)),
        "bq": np.ascontiguousarray(np.asarray(bq, np.float32).reshape(8, 128).T),
        "bk": np.ascontiguousarray(np.asarray(bk, np.float32).reshape(8, 128).T),
        "bv": np.ascontiguousarray(np.asarray(bv, np.float32).reshape(16, 64).T),
        "bo": np.ascontiguousarray(
            np.broadcast_to(np.asarray(bo, np.float32), (128, D))),
    }

    in_maps = []
    for c in range(N_CORES):
        n, half = c // 2, c % 2
        m = dict(w_packed)
        m["xq"] = _pack_kxm(
            np.ascontiguousarray(queries[n, half * LQ:(half + 1) * LQ, :].T))
        m["xk"] = _pack_kxm(np.ascontiguousarray(keys[n].T))
        m["xv"] = _pack_kxm(np.ascontiguousarray(values[n].T))
        in_maps.append(m)

    last_results = run_bass_kernel_spmd(nc, in_maps, list(range(N_CORES)))

    full = np.empty((N, L, D), np.float32)
    for c in range(N_CORES):
        n, half = c // 2, c % 2
        full[n, half * LQ:(half + 1) * LQ, :] = last_results.results[c]["out"]
    return full


# revision 6
# speedup vs baseline: 1.4901x; 1.0354x over previous
"""Multi-head attention layer (N=4, L=S=2048, D=1024, H=16) on 8 TRN2 NeuronCores.

Sharding: 8 cores = 4 batches x 2 query-halves (heads kept local, so no
collectives: each core computes Q projection for its 1024 query rows, K/V
projections for the full 2048 keys of its batch, all 16 heads of attention,
and the output projection for its rows). Host shards/gathers.

Per-core data layout (host-prepared, bf16):
  xq [128, 8, 1024]  xq[p,t,l] = queries[n, l0+l, t*128+p]   (transposed)
  xk/xv [128, 8, 2048]  keys[n].T / values[n].T, same packing
  wq/wk/wv/wo [128, 8, 1024]  w[p,t,d] = W[t*128+p, d]
  bq/bk [128, 8] f32; bv [64, 16] f32; bo [128, 1024] f32 (pre-broadcast)
  out [1024, 1024] f32 (natural layout)

Pipeline notes: attention is ScalarE(exp)-paced — exp runs 1 elem/lane/cycle
at 1.2 GHz vs warm PE at 2.4 GHz, so exp is batched over [128,1024] PSUM
groups (amortizing the 352-cycle ACT overhead) and independent projection
matmul groups are interleaved into the attention emission order as PE
filler, keeping the PE dense so HAM doesn't re-throttle it to 1.2 GHz.
The softmax denominator comes free from a ones-column appended to V (PV
matmul has 65 output rows; row 64 = sum of exp). attn rows sum to 1, so
V's bias is added after normalization (P@(V+bv) = P@V + bv).
"""

import numpy as np
import ml_dtypes

import concourse.bass as bass
import concourse.mybir as mybir
import concourse.tile as tile
from concourse import bacc
from concourse.bass_utils import run_bass_kernel_spmd

BF16 = mybir.dt.bfloat16
F32 = mybir.dt.float32
ALU = mybir.AluOpType
ACTF = mybir.ActivationFunctionType

N, L, S, D, H, E = 4, 2048, 2048, 1024, 16, 64
LQ = 1024
N_CORES = 8

_nc_cache = None
last_results = None


def _build():
    nc = bacc.Bacc(None, target_bir_lowering=False)

    xq = nc.declare_dram_parameter("xq", [128, 8, LQ], BF16, isOutput=False)
    xk = nc.declare_dram_parameter("xk", [128, 8, S], BF16, isOutput=False)
    xv = nc.declare_dram_parameter("xv", [128, 8, S], BF16, isOutput=False)
    wq = nc.declare_dram_parameter("wq", [128, 8, D], BF16, isOutput=False)
    wk = nc.declare_dram_parameter("wk", [128, 8, D], BF16, isOutput=False)
    wv = nc.declare_dram_parameter("wv", [128, 8, D], BF16, isOutput=False)
    wo = nc.declare_dram_parameter("wo", [128, 8, D], BF16, isOutput=False)
    bq = nc.declare_dram_parameter("bq", [128, 8], F32, isOutput=False)
    bk = nc.declare_dram_parameter("bk", [128, 8], F32, isOutput=False)
    bv = nc.declare_dram_parameter("bv", [64, 16], F32, isOutput=False)
    bo = nc.declare_dram_parameter("bo", [128, D], F32, isOutput=False)
    out = nc.declare_dram_parameter("out", [LQ, D], F32, isOutput=True)

    with tile.TileContext(nc) as tc:
        with tc.tile_pool(name="const", bufs=1) as cpool, \
             tc.tile_pool(name="pers", bufs=1) as ppool, \
             tc.tile_pool(name="stage", bufs=2) as spool, \
             tc.tile_pool(name="work", bufs=2) as wpool, \
             tc.tile_pool(name="expp", bufs=3) as epool, \
             tc.tile_pool(name="psum", bufs=2, space="PSUM") as psum:

            # weights: wq's slot is reused by wo after the Q projection.
            wq_t = cpool.tile([128, 8, D], BF16, tag="w_a")
            wk_t = cpool.tile([128, 8, D], BF16, tag="w_b")
            wv_t = cpool.tile([128, 8, D], BF16, tag="w_c")
            nc.sync.dma_start(wq_t[:], wq[:])
            nc.sync.dma_start(wk_t[:], wk[:])
            nc.sync.dma_start(wv_t[:], wv[:])
            bq_t = cpool.tile([128, 8], F32, tag="bq")
            bk_t = cpool.tile([128, 8], F32, tag="bk")
            bv_t = cpool.tile([64, 16], F32, tag="bv")
            bo_t = cpool.tile([128, D], F32, tag="bo")
            nc.sync.dma_start(bq_t[:], bq[:])
            nc.sync.dma_start(bk_t[:], bk[:])
            nc.sync.dma_start(bv_t[:], bv[:])
            nc.sync.dma_start(bo_t[:], bo[:])

            qT = ppool.tile([128, 8, LQ], BF16, tag="qT")
            kT = ppool.tile([128, 8, S], BF16, tag="kT")
            vaug = ppool.tile([128, 16, 16 * 65], BF16, tag="vaug")
            oT = ppool.tile([128, 8, LQ], BF16, tag="oT")

            for st in range(16):
                v3 = vaug[:, st].rearrange("p (h e) -> p h e", e=65)
                nc.vector.memset(v3[:, :, 64:65], 1.0)

            # one projection PSUM group: 8 accumulating matmuls + epilogue
            def proj_group(w_t, sg_t, dt, dst, bias):
                ps = psum.tile([128, 512], F32, tag="proj")
                for ct in range(8):
                    nc.tensor.matmul(ps[:], w_t[:, ct, dt * 128:(dt + 1) * 128],
                                     sg_t[:, ct, :], start=(ct == 0),
                                     stop=(ct == 7))
                nc.vector.tensor_scalar_add(dst, ps[:], bias)

            def q_proj(lb):
                sg = spool.tile([128, 8, 512], BF16, tag="stage")
                nc.sync.dma_start(sg[:], xq[:, :, lb * 512:(lb + 1) * 512])
                return [
                    (lambda dt=dt, sg=sg, lb=lb: proj_group(
                        wq_t, sg, dt, qT[:, dt, lb * 512:(lb + 1) * 512],
                        bq_t[:, dt:dt + 1]))
                    for dt in range(8)
                ]

            def k_proj(sb):
                sg = spool.tile([128, 8, 512], BF16, tag="stage")
                nc.sync.dma_start(sg[:], xk[:, :, sb * 512:(sb + 1) * 512])
                return [
                    (lambda dt=dt, sg=sg, sb=sb: proj_group(
                        wk_t, sg, dt, kT[:, dt, sb * 512:(sb + 1) * 512],
                        bk_t[:, dt:dt + 1]))
                    for dt in range(8)
                ]

            def v_proj_group(sg_t, stl, st, db):
                ps = psum.tile([128, 512], F32, tag="proj")
                for ct in range(8):
                    nc.tensor.matmul(ps[:], sg_t[:, ct, stl * 128:(stl + 1) * 128],
                                     wv_t[:, ct, db * 512:(db + 1) * 512],
                                     start=(ct == 0), stop=(ct == 7))
                v3 = vaug[:, st].rearrange("p (h e) -> p h e", e=65)
                nc.vector.tensor_copy(
                    v3[:, db * 8:(db + 1) * 8, 0:64],
                    ps[:].rearrange("p (h e) -> p h e", e=64))

            def o_proj_group(lt, db):
                ps = psum.tile([128, 512], F32, tag="proj")
                for ct in range(8):
                    nc.tensor.matmul(ps[:], oT[:, ct, lt * 128:(lt + 1) * 128],
                                     wo_t[:, ct, db * 512:(db + 1) * 512],
                                     start=(ct == 0), stop=(ct == 7))
                ob = wpool.tile([128, 512], F32, tag="outsb")
                nc.vector.tensor_add(ob[:], ps[:],
                                     bo_t[:, db * 512:(db + 1) * 512])
                nc.sync.dma_start(
                    out[lt * 128:(lt + 1) * 128, db * 512:(db + 1) * 512], ob[:])

            # ---- up-front projections: Q(lb0), K(all), V(all) ----
            for g in q_proj(0):
                g()
            for sb in range(4):
                for g in k_proj(sb):
                    g()
            for sb in range(4):
                sg = spool.tile([128, 8, 512], BF16, tag="stage")
                nc.sync.dma_start(sg[:], xv[:, :, sb * 512:(sb + 1) * 512])
                for stl in range(4):
                    for db in range(2):
                        v_proj_group(sg, stl, sb * 4 + stl, db)

            # ---- attention, lb-outer; filler groups interleaved ----
            def attention(h, lb, filler):
                pb = (h % 2) * 64
                dt = h // 2
                qh = qT[pb:pb + 64, dt, lb * 512:(lb + 1) * 512]
                po = psum.tile([128, 512], F32, tag="po")
                for sg_i in range(8):       # s-groups of 2 tiles (256 rows)
                    ps2 = psum.tile([128, 1024], F32, tag="sc2")
                    for k in range(2):
                        st = sg_i * 2 + k
                        nc.tensor.matmul(
                            ps2[:, k * 512:(k + 1) * 512],
                            kT[pb:pb + 64, dt, st * 128:(st + 1) * 128],
                            qh, start=True, stop=True)
                    ep = epool.tile([128, 1024], BF16, tag="ep")
                    nc.scalar.activation(ep[:], ps2[:], ACTF.Exp, scale=0.125)
                    if sg_i == 3 and filler:
                        filler.pop(0)()     # PE filler inside the s-loop
                    for k in range(2):
                        st = sg_i * 2 + k
                        nc.tensor.matmul(
                            po[0:65, :], vaug[:, st, h * 65:(h + 1) * 65],
                            ep[:, k * 512:(k + 1) * 512],
                            start=(st == 0), stop=(st == 15))
                den = wpool.tile([128, 512], F32, tag="rec")
                nc.vector.tensor_copy(den[64:65, :], po[64:65, :])
                den0 = wpool.tile([1, 512], F32, tag="rec0")
                nc.sync.dma_start(den0[0:1, :], den[64:65, :])
                denb = wpool.tile([64, 512], F32, tag="recb")
                nc.gpsimd.partition_broadcast(denb[:], den0[0:1, :])
                recb = wpool.tile([64, 512], F32, tag="recf")
                nc.vector.reciprocal_approx_fast(recb[:], denb[:])
                if pb == 0:
                    dst = oT[0:64, dt, lb * 512:(lb + 1) * 512]
                    nc.vector.tensor_tensor(dst, po[0:64, :], recb[:], ALU.mult)
                    nc.vector.tensor_scalar_add(dst, dst, bv_t[:, h:h + 1])
                else:
                    tmp = wpool.tile([64, 512], BF16, tag="otmp")
                    nc.vector.tensor_tensor(tmp[:], po[0:64, :], recb[:],
                                            ALU.mult)
                    nc.vector.tensor_scalar_add(tmp[:], tmp[:], bv_t[:, h:h + 1])
                    nc.sync.dma_start(
                        oT[64:128, dt, lb * 512:(lb + 1) * 512], tmp[:])

            # lb=0 phase: filler = Q projection for lb=1
            filler = q_proj(1)
            for h in range(16):
                attention(h, 0, filler)
            while filler:
                filler.pop(0)()
            wo_t = cpool.tile([128, 8, D], BF16, tag="w_a")  # reuses wq slot
            nc.sync.dma_start(wo_t[:], wo[:])

            # lb=1 phase: filler = output projection for l-tiles 0..3
            filler = [
                (lambda lt=lt, db=db: o_proj_group(lt, db))
                for lt in range(4) for db in range(2)
            ]
            for h in range(16):
                attention(h, 1, filler)
            while filler:
                filler.pop(0)()

            for lt in range(4, 8):
                for db in range(2):
                    o_proj_group(lt, db)

    nc.compile()
    return nc


def _pack_kxm(w):
    k, m = w.shape
    return np.ascontiguousarray(
        w.reshape(k // 128, 128, m).transpose(1, 0, 2)).astype(ml_dtypes.bfloat16)


def kernel(queries, keys, values, Wq, bq, Wk, bk, Wv, bv, Wo, bo):
    global _nc_cache, last_results
    queries = np.asarray(queries, dtype=np.float32)
    keys = np.asarray(keys, dtype=np.float32)
    values = np.asarray(values, dtype=np.float32)

    if _nc_cache is None:
        _nc_cache = _build()
    nc = _nc_cache

    w_packed = {
        "wq": _pack_kxm(np.asarray(Wq, np.float32)),
        "wk": _pack_kxm(np.asarray(Wk, np.float32)),
        "wv": _pack_kxm(np.asarray(Wv, np.float32)),
        "wo": _pack_kxm(np.asarray(Wo, np.float32)),
        "bq": np.ascontiguousarray(np.asarray(bq, np.float32).reshape(8, 128).T),
        "bk": np.ascontiguousarray(np.asarray(bk, np.float32).reshape(8, 128).T),
        "bv": np.ascontiguousarray(np.asarray(bv, np.float32).reshape(16, 64).T),
        "bo": np.ascontiguousarray(
            np.broadcast_to(np.asarray(bo, np.float32), (128, D))),
    }

    in_maps = []
    for c in range(N_CORES):
        n, half = c // 2, c % 2
        m = dict(w_packed)
        m["xq"] = _pack_kxm(
            np.ascontiguousarray(queries[n, half * LQ:(half + 1) * LQ, :].T))
        m["xk"] = _pack_kxm(np.ascontiguousarray(keys[n].T))
        m["xv"] = _pack_kxm(np.ascontiguousarray(values[n].T))
        in_maps.append(m)

    last_results = run_bass_kernel_spmd(nc, in_maps, list(range(N_CORES)))

    full = np.empty((N, L, D), np.float32)
    for c in range(N_CORES):
        n, half = c // 2, c % 2
        full[n, half * LQ:(half + 1) * LQ, :] = last_results.results[c]["out"]
    return full
